# revision 5
# baseline (speedup 1.0000x reference)
"""Exponential smoothing (linear recurrence scan) on 8 trn2 NeuronCores.

Math (per batch b, head h, dim d):
    alpha = sigmoid(smoothing_weight[h])
    u[t]  = (1-alpha)*values[t] + factor*alpha*aux_values[t]
    y[t]  = alpha*y[t-1] + u[t],   y[-1] = v0
Sharding: data-parallel over batch b -> 8 cores, one batch each.

Device algorithm (per core, T=4096, HD=H*D=512), all fp32 data:
  - T is split into NG groups x CPG chunks of 128 rows.
  - Main: per head, Y_local = (c1*L)^T-matmul(v) + (c2*L)^T-matmul(a) where
    L[p,q] = alpha^(p-q) (p>=q) is the within-chunk scan matrix.  Chunks are
    batched along the matmul free dim (float32r -> 1 cycle/row at N>=256).
  - Level-2: chunk summaries s_c = Y_local[c][127] are scanned across chunks
    with small per-head matmuls using A = alpha^128 power matrices, yielding
    the carry P_c = S_{c-1} entering each chunk (S_{-1} = v0).
  - Fixup: rank-1 matmul decay_h (x) P_row accumulated into the output:
    y[128c+p] = Y_local[c][p] + alpha^(p+1) * P_c.
Cross-partition moves (chunk summaries -> partition-per-chunk, carries ->
single row) are done with small SBUF->SBUF DMAs.
"""

import sys

sys.path.insert(0, "/opt/trn_rl_repo")

import numpy as np

import concourse.bass as bass
import concourse.bacc as bacc
import concourse.mybir as mybir
from concourse.tile import TileContext
from concourse.bass_utils import run_bass_kernel_spmd

B, T, H, D = 8, 4096, 8, 64
HD = H * D                  # 512
P = 128                     # chunk length / partitions
NCHUNK = T // P             # 32
CPG = 8                     # chunks per group
NG = NCHUNK // CPG          # 4 groups
GT = CPG * P                # 1024 rows per group

F32 = mybir.dt.float32
F32R = mybir.dt.float32r


def build_consts(smoothing_weight, v0):
    """Host-side constant tensors (float64 math, cast to fp32)."""
    a = 1.0 / (1.0 + np.exp(-smoothing_weight.astype(np.float64).reshape(H)))
    c1 = 1.0 - a
    factor = c1 / np.maximum(c1, 1e-6)
    c2 = factor * a

    q = np.arange(P)
    e = q[None, :] - q[:, None]                     # [q, p] -> p - q
    pow_ = np.where(e >= 0, a[:, None, None] ** np.maximum(e, 0), 0.0)  # [h,q,p]
    w1 = (c1[:, None, None] * pow_).transpose(1, 0, 2).reshape(P, H * P)
    w2 = (c2[:, None, None] * pow_).transpose(1, 0, 2).reshape(P, H * P)

    decay = (a[:, None] ** (q[None, :] + 1)).reshape(1, H * P)  # [1, h*128]

    A = a ** P                                       # alpha^128 per head
    r = np.arange(CPG + 1)
    # MgT[j, r] = A^(r-1-j) for j <= r-1 else 0   (lhsT of the level-2 scan)
    ee = (r[None, :] - 1) - np.arange(CPG)[:, None]  # [j, r]
    mexc = np.where(ee >= 0, A[:, None, None] ** np.maximum(ee, 0), 0.0)  # [h,j,r]
    mexc = mexc.transpose(1, 0, 2).reshape(CPG, H * (CPG + 1))
    avec = (A[:, None] ** r[None, :]).reshape(1, H * (CPG + 1))

    v0row = v0.astype(np.float64).reshape(1, HD)

    f = np.float32
    return {
        "w1": np.ascontiguousarray(w1, dtype=f),
        "w2": np.ascontiguousarray(w2, dtype=f),
        "decay": np.ascontiguousarray(decay, dtype=f),
        "mexc": np.ascontiguousarray(mexc, dtype=f),
        "avec": np.ascontiguousarray(avec, dtype=f),
        "v0r": np.ascontiguousarray(v0row, dtype=f),
    }


def build_nc():
    nc = bacc.Bacc()

    v_d = nc.declare_dram_parameter("v", [T, HD], F32R, isOutput=False)
    a_d = nc.declare_dram_parameter("a", [T, HD], F32R, isOutput=False)
    w1_d = nc.declare_dram_parameter("w1", [P, H * P], F32R, isOutput=False)
    w2_d = nc.declare_dram_parameter("w2", [P, H * P], F32R, isOutput=False)
    dec_d = nc.declare_dram_parameter("decay", [1, H * P], F32R, isOutput=False)
    mex_d = nc.declare_dram_parameter("mexc", [CPG, H * (CPG + 1)], F32, isOutput=False)
    av_d = nc.declare_dram_parameter("avec", [1, H * (CPG + 1)], F32, isOutput=False)
    v0_d = nc.declare_dram_parameter("v0r", [1, HD], F32, isOutput=False)
    y_d = nc.declare_dram_parameter("y", [T, HD], F32, isOutput=True)

    with TileContext(nc) as tc:
        with (
            tc.tile_pool(name="wpool", bufs=1) as wpool,
            tc.tile_pool(name="vin", bufs=2) as vin,
            tc.tile_pool(name="ain", bufs=2) as ain,
            tc.tile_pool(name="yout", bufs=2) as yout,
            tc.tile_pool(name="small", bufs=2) as small,
            tc.tile_pool(name="psA", bufs=4, space="PSUM") as psA_pool,
            tc.tile_pool(name="psP", bufs=2, space="PSUM") as psP_pool,
            tc.tile_pool(name="psB", bufs=2, space="PSUM") as psB_pool,
        ):
            # constants -> SBUF once
            w1 = wpool.tile([P, H * P], F32R, tag="w1")
            w2 = wpool.tile([P, H * P], F32R, tag="w2")
            dec = wpool.tile([1, H * P], F32R, tag="dec")
            mex = wpool.tile([CPG, H * (CPG + 1)], F32, tag="mex")
            av = wpool.tile([1, H * (CPG + 1)], F32, tag="av")
            v0r = wpool.tile([1, HD], F32, tag="v0r")
            nc.sync.dma_start(w1[:], w1_d[:])
            nc.sync.dma_start(w2[:], w2_d[:])
            nc.sync.dma_start(dec[:], dec_d[:])
            nc.sync.dma_start(mex[:], mex_d[:])
            nc.sync.dma_start(av[:], av_d[:])
            nc.sync.dma_start(v0r[:], v0_d[:])

            carry = v0r  # [1, HD] carry entering group 0

            for g in range(NG):
                r0 = g * GT
                # ---- stream group inputs:  [GT, HD] -> [P, CPG, HD]
                v_sb = vin.tile([P, CPG * HD], F32R, tag="v")
                a_sb = ain.tile([P, CPG * HD], F32R, tag="a")
                src_v = v_d[r0:r0 + GT, :].rearrange("(c p) m -> p c m", c=CPG, p=P)
                src_a = a_d[r0:r0 + GT, :].rearrange("(c p) m -> p c m", c=CPG, p=P)
                nc.sync.dma_start(v_sb[:].rearrange("p (c m) -> p c m", c=CPG), src_v)
                nc.sync.dma_start(a_sb[:].rearrange("p (c m) -> p c m", c=CPG), src_a)

                y_sb = yout.tile([P, CPG * HD], F32, tag="y")
                v3 = v_sb[:].rearrange("p (c m) -> p c m", c=CPG)
                a3 = a_sb[:].rearrange("p (c m) -> p c m", c=CPG)
                y3 = y_sb[:].rearrange("p (c m) -> p c m", c=CPG)

                # ---- main within-chunk scan, per head
                for h in range(H):
                    psA = psA_pool.tile([P, CPG * D], F32, tag="psA")
                    rhs_v = v3[:, :, h * D:(h + 1) * D]   # [P, CPG, D]
                    rhs_a = a3[:, :, h * D:(h + 1) * D]
                    nc.tensor.matmul(psA[:], w1[:, h * P:(h + 1) * P], rhs_v,
                                     start=True, stop=False)
                    nc.tensor.matmul(psA[:], w2[:, h * P:(h + 1) * P], rhs_a,
                                     start=False, stop=True)
                    # evacuate into interleaved (c, h, d) layout
                    nc.scalar.copy(y3[:, :, h * D:(h + 1) * D],
                                   psA[:].rearrange("p (c d) -> p c d", c=CPG))

                # ---- gather chunk summaries: row 127 -> [CPG, HD]
                sT = small.tile([CPG, HD], F32, tag="sT")
                nc.gpsimd.dma_start(sT[:], y_sb[P - 1:P, :])

                # ---- level-2 scan across chunks (tiny matmuls per head)
                psP = psP_pool.tile([CPG + 1, HD], F32, tag="psP")
                for h in range(H):
                    hs = slice(h * (CPG + 1), (h + 1) * (CPG + 1))
                    nc.tensor.matmul(psP[:, h * D:(h + 1) * D], mex[:, hs],
                                     sT[:, h * D:(h + 1) * D],
                                     start=True, stop=False)
                    nc.tensor.matmul(psP[:, h * D:(h + 1) * D], av[:, hs],
                                     carry[0:1, h * D:(h + 1) * D],
                                     start=False, stop=True)
                p_sb = small.tile([CPG + 1, HD], F32, tag="p_sb")
                nc.scalar.copy(p_sb[:], psP[:])

                # ---- scatter carries to a single row [1, (c, h, d)]
                prow = small.tile([1, CPG * HD], F32, tag="prow")
                nc.gpsimd.dma_start(prow[:], p_sb[0:CPG, :])
                carry_next = small.tile([1, HD], F32, tag="carry")
                nc.gpsimd.dma_start(carry_next[:], p_sb[CPG:CPG + 1, :])

                # ---- fixup: rank-1 decay (x) carry, then add into y
                prow3 = prow[:].rearrange("o (c m) -> o c m", c=CPG)
                for h in range(H):
                    psB = psB_pool.tile([P, CPG * D], F32, tag="psB")
                    nc.tensor.matmul(psB[:], dec[0:1, h * P:(h + 1) * P],
                                     prow3[:, :, h * D:(h + 1) * D].bitcast(F32R),
                                     start=True, stop=True)
                    yv = y3[:, :, h * D:(h + 1) * D]
                    nc.vector.tensor_add(
                        yv, yv, psB[:].rearrange("p (c d) -> p c d", c=CPG))

                # ---- store group output
                dst = y_d[r0:r0 + GT, :].rearrange("(c p) m -> p c m", c=CPG, p=P)
                nc.scalar.dma_start(dst, y_sb[:].rearrange("p (c m) -> p c m", c=CPG))

                carry = carry_next

    nc.finalize()
    return nc


_NC_CACHE = None


def _get_nc():
    global _NC_CACHE
    if _NC_CACHE is None:
        _NC_CACHE = build_nc()
    return _NC_CACHE


def kernel(values, aux_values, v0, smoothing_weight):
    consts = build_consts(smoothing_weight, v0)
    nc = _get_nc()
    in_maps = []
    for b in range(B):
        m = dict(consts)
        m["v"] = np.ascontiguousarray(values[b].reshape(T, HD), dtype=np.float32)
        m["a"] = np.ascontiguousarray(aux_values[b].reshape(T, HD), dtype=np.float32)
        in_maps.append(m)
    res = run_bass_kernel_spmd(nc, in_maps, list(range(B))).results
    out = np.stack([res[b]["y"].reshape(T, H, D) for b in range(B)])
    return out.astype(np.float32)
